# revision 41
# baseline (speedup 1.0000x reference)
"""EngagementBiasedMHA on 8 Trainium2 NeuronCores.

Sharding: 4 batches x 2 head-groups (8 heads each).  Each core computes, for
its (batch, head-group):
  - K^T projection in [feat, token] layout and V projection in [token, feat]
    layout; V is stored per key-tile as [ones(64) | V_h] so the PV
    matmul also produces the softmax denominator on partitions 0:64
  - per 512-query chunk: attention in transposed layout: S^T = K @ Q^T with
    keys on partitions, so the per-key engagement bias/mask folds into the
    Exp activation as a per-partition bias, and exp(S^T) is already the
    correct (lhs-contraction) layout for the PV matmul
  - O^T = Vhat^T @ P^T accumulated over key tiles (rows 0:64 = replicated
    softmax denominator, rows 64:128 = head output)
  - row-parallel partial output projection y_partial = O_hg @ out_w.T[hg],
    written back in bf16 (host sums the two partials per batch in fp32)

Schedule: steady state alternates one exp of [128,1024] per period with a PE
budget of [lagged PV pair, ~1 filler unit, S^T pair last].  The S^T pair sits
at the END of each period so its WAR on the exp two periods back never blocks
the in-order PE queue.  Projections (Q/K/V/out) are EDF-packed into per-period
filler slots against their JIT deadlines.  The engagement bias ln() is folded
on the host; the exp activation table is preloaded at t=0 by a dummy exp.
"""

import sys

if "/opt/trn_rl_repo" not in sys.path:
    sys.path.insert(0, "/opt/trn_rl_repo")

import numpy as np
from concourse import bacc, tile
import concourse.mybir as mybir
from concourse.bass_utils import run_bass_kernel_spmd

F32 = mybir.dt.float32
BF16 = mybir.dt.bfloat16
NP_BF16 = mybir.dt.np(BF16)
AF = mybir.ActivationFunctionType

B, T, D, H = 4, 2048, 1024, 16
HD = 64
HG = 8           # heads per core
NKT = T // 128   # 16 key/token tiles
NQC = T // 512   # 4 query chunks
NDT = D // 128   # 8 d_in tiles
VROW = HG * 128  # 1024 Vhat columns per key tile: per head [ones(64) | V(64)]

_cache = {}

# Results of the most recent run (for the test harness to read exec times).
last_results = None

# Block order: all (qc, hp0/hp1) first, then all (qc, hp2/hp3).  This way the
# K projections for hp2/hp3 (m=6,7) and the V projections for heads 4..7 have
# 6 light blocks (2..7) to hide in, instead of crowding the first blocks, and
# each qc still completes early enough to spread its output projection.
BLOCKS = [(0, 0), (0, 1), (1, 0), (1, 1), (2, 0), (2, 1), (3, 0), (3, 1),
          (0, 2), (0, 3), (1, 2), (1, 3), (2, 2), (2, 3), (3, 2), (3, 3)]
NB = len(BLOCKS)
FIRST_HP_BLOCK = {0: 0, 1: 1, 2: 8, 3: 9}   # first block using head-pair hp
QC_DONE_BLOCK = {0: 9, 1: 11, 2: 13, 3: 15}  # last block of each qc


def _build_schedule():
    """Jointly EDF-pack pv pairs and projection fillers into periods.

    Costs are in u = 1024 matmul columns (~427ns warm).  Every period also
    carries an S^T pair (0.5u) implicitly.  Item kinds:
      ('pv', p_src)       : 1u, the lagged PV pair for period p_src
      ('v', hpair, kt)    : 1u, 8-matmul N=128 proj_v unit
      ('qk', m, c)        : 4u chain -> two 2u halves in consecutive periods
      ('outp', qc, t4)    : 4u chain -> two 2u halves (shared yv tile)
      ('f01', t4, c2)     : 1u, qc3 partial (2 matmuls)
    psmix slot discipline: at most one chain-open per period, at most two
    mix-opens total per period, chains span exactly 2 periods.
    Returns dict period -> ordered list of (key, part).
    """
    NP_ = NB * NKT
    HORIZON = NP_ + 4   # late pv pairs of the last block land past the loop

    def blk_start(bi):
        return bi * NKT

    items = []   # [deadline, earliest, cost, key]
    # pv pairs: window [src + lag_lo, min(src+12, next block start + 3)],
    # also not before the op-slot is freed by the previous block's tail
    # (emitted at (bi+1, kt4)); late kts get a shallow lag so they finish
    # before that tail.
    for bi in range(NB):
        for kt in range(NKT):
            src = bi * NKT + kt
            lag_lo = 4 if kt < 12 else 3
            earliest = max(src + lag_lo, blk_start(bi) + 5)
            deadline = min(src + 12, blk_start(bi + 1) + 3)
            assert earliest <= deadline, (bi, kt)
            items.append([deadline, earliest, 1.0, ('pv', src)])
    # K feature chains: m=4+hp, chunk c needed by (first hp block, kt=4c)
    for hp in range(4):
        for c in range(4):
            if hp == 0 and c == 0:
                continue  # prologue
            dl = blk_start(FIRST_HP_BLOCK[hp]) + 4 * c - 2
            items.append([max(dl, 1), 0, 4.0, ('qk', 4 + hp, c)])
    # Q feature chains: m=hp, c=qc needed by block start
    for bi in range(1, NB):
        qc, hp = BLOCKS[bi]
        items.append([blk_start(bi) - 2, 0, 4.0, ('qk', hp, qc)])
    # V units: needed by PV(first block of the head pair, kt).  Heads 0-3
    # are JIT for blocks 0/1 and stay 2-head (1u) for schedule granularity;
    # heads 4-7 have slack until block 8 and go as 4-head N=256 units
    # (slightly better PE efficiency).
    for hp in range(2):
        for kt in range(NKT):
            dl = blk_start(FIRST_HP_BLOCK[hp]) + max(kt + 2, 3)
            items.append([dl, 0, 1.0, ('v', hp, kt)])
    for kt in range(NKT):
        dl = blk_start(FIRST_HP_BLOCK[2]) + max(kt + 2, 3)
        items.append([dl, 0, 2.0, ('v4', 1, kt)])
    # out-proj chains + qc3 partials
    for qc in range(4):
        for t4 in range(4):
            if qc < 3:
                rdy = blk_start(QC_DONE_BLOCK[qc] + 1) + 4
                items.append([NP_ - 1, rdy, 4.0, ('outp', qc, t4)])
            else:
                for c2 in range(2):
                    items.append([NP_ - 1, blk_start(8) + 4, 1.0,
                                  ('f01', t4, c2)])
                    # hp2's contribution folds into the partial during the
                    # last block, leaving only hp3 + writeback for the tail
                    items.append([NP_ - 1, blk_start(15) + 5, 0.5,
                                  ('f2', t4, c2)])

    fill_cap = [2.00] * HORIZON
    # 3.05 lets one pv pair share a chain period, smearing the pv pile-ups
    # at block-boundary deadline walls
    tot_cap = [3.05] * HORIZON
    for p in range(2 * NKT):
        fill_cap[p] = 4.00      # blocks 0-1 carry the startup JIT
        tot_cap[p] = 5.00
    for p in range(NP_, HORIZON):
        fill_cap[p] = 6.00
        tot_cap[p] = 8.00
    sched = {p: [] for p in range(HORIZON)}
    load = [0.0] * HORIZON
    opens = [0] * HORIZON
    chain_open = [False] * HORIZON
    reserved = [False] * HORIZON    # second half of a chain already placed

    # pass 1: projection fillers (EDF); pass 2: pv pairs into the gaps.
    # V and Q/K work is held back to ~24 periods before its deadline so the
    # mid-kernel valley (after the startup JIT, before out-proj readiness)
    # gets backfilled instead of everything cramming into the first blocks.
    for it in items:
        if it[3][0] in ('v', 'qk'):
            it[1] = max(it[1], it[0] - 24)
    fillers = sorted((it for it in items if it[3][0] != 'pv'),
                     key=lambda it: (it[0], it[3]))
    pending = list(fillers)
    for p in range(HORIZON):
        i = 0
        while i < len(pending) and load[p] < fill_cap[p]:
            dl, earliest, cost, key = pending[i]
            kind = key[0]
            if p < earliest:
                i += 1
                continue
            if kind in ('qk', 'outp'):
                ok = (p + 1 < HORIZON
                      and load[p] + 2.0 <= fill_cap[p] + 0.01
                      and load[p + 1] + 2.0 <= fill_cap[p + 1] + 0.01
                      and not chain_open[p] and not reserved[p]
                      and not chain_open[p + 1]
                      and opens[p] < 2 and opens[p + 1] < 2)
                if ok:
                    sched[p].append((key, 1))
                    sched[p + 1].append((key, 2))
                    load[p] += 2.0
                    load[p + 1] += 2.0
                    chain_open[p] = True
                    reserved[p + 1] = True
                    opens[p] += 1
                    opens[p + 1] += 1
                    pending.pop(i)
                    continue
            else:
                if load[p] + cost <= fill_cap[p] + 0.01 and opens[p] < 2:
                    sched[p].append((key, 0))
                    load[p] += cost
                    opens[p] += 1
                    pending.pop(i)
                    continue
            i += 1
        for it in pending:
            if it[0] <= p:
                it[0] = p + 1   # soft deadline slip
    assert not pending, f"unscheduled fillers: {pending[:6]} ..."

    pvs = sorted((it for it in items if it[3][0] == 'pv'),
                 key=lambda it: it[3][1])   # strict kt order per block
    block_last = [0] * NB
    for it in pvs:
        dl, earliest, cost, key = it
        bi = key[1] // NKT
        lo = max(earliest, block_last[bi])  # keep per-block execution order
        hi = min(dl, HORIZON - 1)
        # first-fit keeps per-block placement monotone and spreads left
        cands = [p for p in range(lo, hi + 1)
                 if load[p] + cost <= tot_cap[p] + 0.01]
        best = (cands[0] if cands
                else min(range(lo, hi + 1), key=lambda p: load[p]))
        sched[best].append((key, 0))
        load[best] += cost
        block_last[bi] = best
    # order within each period: pv first, then fillers (chain halves early)
    for p in range(HORIZON):
        sched[p].sort(key=lambda kp: 0 if kp[0][0] == 'pv' else 1)
    return sched


def _build_program():
    nc = bacc.Bacc("TRN2", target_bir_lowering=False, debug=False, num_devices=8)
    xt_d = nc.declare_dram_parameter("xt", [D, T], BF16, isOutput=False)
    # wqk: row block m*128+p holds, at col d*128+f, weight qkv_w.T[d*128+p, feat(m)+f]
    wqk_d = nc.declare_dram_parameter("wqk", [1024, 1024], BF16, isOutput=False)
    wv_d = nc.declare_dram_parameter("wv", [D, 512], BF16, isOutput=False)
    # small1 = [bqk(8) | BK(16)] merged to one DMA; BK = ln(eng)-1e9*mask (host)
    small1_d = nc.declare_dram_parameter("small1", [128, 24], F32, isOutput=False)
    bv_d = nc.declare_dram_parameter("bv", [128, 512], F32, isOutput=False)
    wo_d = nc.declare_dram_parameter("wo", [512, 1024], BF16, isOutput=False)
    bo_d = nc.declare_dram_parameter("bo", [128, 1024], F32, isOutput=False)
    y_d = nc.declare_dram_parameter("y", [T, D], BF16, isOutput=True)

    sched = _build_schedule()

    with tile.TileContext(nc) as tc:
        with (
            tc.tile_pool(name="persist", bufs=1) as persist,
            tc.tile_pool(name="wvpool", bufs=1) as wvpool,
            tc.tile_pool(name="wopool", bufs=1) as wopool,
            tc.tile_pool(name="small", bufs=1) as small,
            tc.tile_pool(name="ptpool", bufs=13) as ptpool,
            tc.tile_pool(name="otpool", bufs=16) as otpool,
            tc.tile_pool(name="evacpool", bufs=3) as evacpool,
            tc.tile_pool(name="p01pool", bufs=8) as p01pool,
            tc.tile_pool(name="recpool", bufs=3) as recpool,
            tc.tile_pool(name="psmix", bufs=2, space="PSUM") as psmix,
            tc.tile_pool(name="psops", bufs=2, space="PSUM") as psops,
            tc.tile_pool(name="psST", bufs=2, space="PSUM") as psST,
        ):
            # ---- resident activations / weights (bf16) ----
            XT = persist.tile([128, NDT * T], BF16, name="XT")
            WQK = persist.tile([128, 8 * 1024], BF16, name="WQK")
            WV = wvpool.tile([128, NDT * 512], BF16, name="WV")
            WO = wopool.tile([128, 4 * 1024], BF16, name="WO")
            SM1 = small.tile([128, 24], F32, name="SM1")
            BV = small.tile([128, 512], F32, name="BV")
            BO = small.tile([128, 1024], F32, name="BO")
            QTKT = persist.tile([128, 8 * T], BF16, name="QTKT")
            VHAT = persist.tile([128, NKT * VROW], BF16, name="VHAT")
            DUM = small.tile([1, 8], F32, name="DUM")

            BQK = SM1[:, 0:8]
            BK = SM1[:, 8:24]

            # Preload the exp activation table off the critical path: a dummy
            # exp on a tiny tile triggers the ~2.7us ACT_TABLE_LOAD at t~2us.
            nc.vector.memset(DUM[:], 0.0)
            nc.scalar.activation(DUM[:], DUM[:], AF.Exp)
            # ~6us of tiny matmuls warm the PE (HAM releases the 1.2 GHz
            # cold throttle after ~3.4us of activity) so the DMA-bound
            # prologue projections run at the full 2.4 GHz clock.
            for _ in range(120):
                wps = psmix.tile([8, 8], F32, name="wps", tag="mix")
                nc.tensor.matmul(wps[:], lhsT=DUM[:], rhs=DUM[:],
                                 start=True, stop=True)

            def dma_wqk(eng, m, splits=1):
                # split across partition ranges -> parallel DMA queues
                step = 128 // splits
                for s in range(splits):
                    eng.dma_start(
                        WQK[s * step:(s + 1) * step, m * 1024:(m + 1) * 1024],
                        wqk_d[m * 128 + s * step: m * 128 + (s + 1) * step, :])

            def dma_xt_chunk(eng, c):
                for d in range(NDT):
                    eng.dma_start(
                        XT[:, d * T + c * 512: d * T + (c + 1) * 512],
                        xt_d[d * 128:(d + 1) * 128, c * 512:(c + 1) * 512])

            # DMA dispatch lanes (all 2D contiguous descriptors -> hardware
            # DGE; 3D/rearranged APs fall back to software DGE, which was
            # measured to downclock the whole chip ~20%).  The exp-critical
            # transfers (SM1, WQK m4/m0, XT chunk 0) lead the sync and
            # gpsimd queues; scalar dispatches two late weights early on.
            def dma_xt_part(eng, c, dlo, dhi):
                for d in range(dlo, dhi):
                    eng.dma_start(
                        XT[:, d * T + c * 512: d * T + (c + 1) * 512],
                        xt_d[d * 128:(d + 1) * 128, c * 512:(c + 1) * 512])

            nc.sync.dma_start(SM1[:], small1_d[:])
            dma_wqk(nc.sync, 4)
            dma_xt_part(nc.sync, 0, 0, 3)
            dma_wqk(nc.sync, 0)
            dma_xt_part(nc.sync, 1, 0, 8)
            dma_xt_part(nc.sync, 2, 0, 8)
            dma_xt_part(nc.sync, 3, 0, 8)
            dma_wqk(nc.sync, 5)
            dma_wqk(nc.sync, 1)

            dma_xt_part(nc.gpsimd, 0, 3, 8)
            for d in range(NDT):
                nc.gpsimd.dma_start(WV[:, d * 512:(d + 1) * 512],
                                    wv_d[d * 128:(d + 1) * 128, :])
            nc.gpsimd.dma_start(BV[:], bv_d[:])
            dma_wqk(nc.gpsimd, 6)
            dma_wqk(nc.gpsimd, 7)
            nc.gpsimd.dma_start(BO[:], bo_d[:])
            for f in range(4):
                nc.gpsimd.dma_start(WO[:, f * 1024:(f + 1) * 1024],
                                    wo_d[f * 128:(f + 1) * 128, :])

            dma_wqk(nc.scalar, 2)
            dma_wqk(nc.scalar, 3)

            # VHAT ones memsets: kt 0-5 early on DVE (block 0 needs them),
            # the rest on gpsimd once its DMA dispatches drain.
            for t in range(6):
                nc.vector.memset(VHAT[:, t * VROW:(t + 1) * VROW], 1.0)
            for t in range(6, NKT):
                nc.gpsimd.memset(VHAT[:, t * VROW:(t + 1) * VROW], 1.0)

            # ---- projection helpers (bias-add on DVE, not ACT) ----
            qk_state = {}

            def proj_qk_part(m, c, dlo, dhi):
                if dlo == 0:
                    ps = psmix.tile([128, 512], F32, name="ps_qk", tag="mix")
                    qk_state[(m, c)] = ps
                else:
                    ps = qk_state[(m, c)]
                for d in range(dlo, dhi):
                    nc.tensor.matmul(
                        ps[:],
                        lhsT=WQK[:, m * 1024 + d * 128: m * 1024 + (d + 1) * 128],
                        rhs=XT[:, d * T + c * 512: d * T + c * 512 + 512],
                        start=(d == 0), stop=(d == NDT - 1),
                    )
                if dhi == NDT:
                    del qk_state[(m, c)]
                    nc.vector.tensor_scalar_add(
                        QTKT[:, m * T + c * 512: m * T + c * 512 + 512],
                        ps[:], BQK[:, m:m + 1])

            def proj_v2(hpair, kt):
                # V feats for heads (2*hpair, 2*hpair+1), token tile kt
                h0 = 2 * hpair
                ps = psmix.tile([128, 128], F32, name="ps_v", tag="mix")
                for d in range(NDT):
                    nc.tensor.matmul(
                        ps[:],
                        lhsT=XT[:, d * T + kt * 128: d * T + (kt + 1) * 128],
                        rhs=WV[:, d * 512 + h0 * 64: d * 512 + (h0 + 2) * 64],
                        start=(d == 0), stop=(d == NDT - 1),
                    )
                vslice = VHAT[:, kt * VROW + h0 * 128: kt * VROW + (h0 + 2) * 128
                              ].rearrange("p (h c) -> p h c", c=128)[:, :, 64:128]
                nc.vector.tensor_add(
                    vslice,
                    ps[:].rearrange("p (h c) -> p h c", c=64),
                    BV[:, h0 * 64:(h0 + 2) * 64].rearrange(
                        "p (h c) -> p h c", c=64))

            def proj_v4(pair, kt):
                # V feats for heads [4*pair, 4*pair+4), token tile kt (N=256)
                ps = psmix.tile([128, 256], F32, name="ps_v4", tag="mix")
                for d in range(NDT):
                    nc.tensor.matmul(
                        ps[:],
                        lhsT=XT[:, d * T + kt * 128: d * T + (kt + 1) * 128],
                        rhs=WV[:, d * 512 + pair * 256: d * 512 + (pair + 1) * 256],
                        start=(d == 0), stop=(d == NDT - 1),
                    )
                vslice = VHAT[:, kt * VROW + pair * 512: kt * VROW + (pair + 1) * 512
                              ].rearrange("p (h c) -> p h c", c=128)[:, :, 64:128]
                nc.vector.tensor_add(
                    vslice,
                    ps[:].rearrange("p (h c) -> p h c", c=64),
                    BV[:, pair * 256:(pair + 1) * 256].rearrange(
                        "p (h c) -> p h c", c=64))

            # ---- output projection ----
            otc_by_qc = {}
            yv_tiles = {}
            p01_tiles = {}
            dma_flip = [0]

            def y_dma(qc, t4, yv):
                tt = qc * 4 + t4
                eng = nc.sync if dma_flip[0] % 2 == 0 else nc.gpsimd
                dma_flip[0] += 1
                eng.dma_start(y_d[tt * 128:(tt + 1) * 128, :], yv[:])

            def out_half(qc, t4, c2):
                otc = otc_by_qc[qc]
                ps = psmix.tile([128, 512], F32, name="ps_y", tag="mix")
                for f in range(4):
                    nc.tensor.matmul(
                        ps[:],
                        lhsT=otc[f][:, t4 * 128:(t4 + 1) * 128],
                        rhs=WO[:, f * 1024 + c2 * 512: f * 1024 + c2 * 512 + 512],
                        start=(f == 0), stop=(f == 3))
                key = (qc, t4)
                if key not in yv_tiles:
                    yv_tiles[key] = evacpool.tile([128, 1024], BF16, name="yv",
                                                  tag="yv")
                    first = True
                else:
                    first = False
                yv = yv_tiles[key]
                nc.vector.tensor_add(yv[:, c2 * 512:(c2 + 1) * 512], ps[:],
                                     BO[:, c2 * 512:(c2 + 1) * 512])
                if not first:
                    y_dma(qc, t4, yv_tiles.pop(key))

            def out_f01(t4, c2):
                # qc3 partial: contributions of head-pairs 0,1 (+ bias)
                otc = otc_by_qc[3]
                ps = psmix.tile([128, 512], F32, name="ps_y", tag="mix")
                for f in range(2):
                    nc.tensor.matmul(
                        ps[:],
                        lhsT=otc[f][:, t4 * 128:(t4 + 1) * 128],
                        rhs=WO[:, f * 1024 + c2 * 512: f * 1024 + c2 * 512 + 512],
                        start=(f == 0), stop=(f == 1))
                p01 = p01pool.tile([128, 512], F32, name="p01", tag="p01")
                p01_tiles[(t4, c2)] = p01
                nc.vector.tensor_add(p01[:], ps[:], BO[:, c2 * 512:(c2 + 1) * 512])

            def out_f2(t4, c2):
                # fold hp2's contribution into the partial during block 15
                otc = otc_by_qc[3]
                ps = psmix.tile([128, 512], F32, name="ps_y", tag="mix")
                nc.tensor.matmul(
                    ps[:],
                    lhsT=otc[2][:, t4 * 128:(t4 + 1) * 128],
                    rhs=WO[:, 2 * 1024 + c2 * 512: 2 * 1024 + c2 * 512 + 512],
                    start=True, stop=True)
                p01 = p01_tiles[(t4, c2)]
                nc.vector.tensor_add(p01[:], ps[:], p01[:])

            def out_f3(t4, c2, k):
                # epilogue: rotate across psmix AND the now-idle psST banks
                # so the matmuls never wait on the DVE adds
                otc = otc_by_qc[3]
                if k % 2 == 0:
                    ps = psmix.tile([128, 512], F32, name="ps_y", tag="mix")
                else:
                    ps = psST.tile([128, 512], F32, name="ps_y2", tag="st")
                nc.tensor.matmul(
                    ps[:],
                    lhsT=otc[3][:, t4 * 128:(t4 + 1) * 128],
                    rhs=WO[:, 3 * 1024 + c2 * 512: 3 * 1024 + c2 * 512 + 512],
                    start=True, stop=True)
                key = (3, t4)
                if key not in yv_tiles:
                    yv_tiles[key] = evacpool.tile([128, 1024], BF16, name="yv",
                                                  tag="yv")
                    first = True
                else:
                    first = False
                yv = yv_tiles[key]
                nc.vector.tensor_add(yv[:, c2 * 512:(c2 + 1) * 512], ps[:],
                                     p01_tiles.pop((t4, c2))[:])
                if not first:
                    # split the final writebacks across both DMA lanes
                    yv = yv_tiles.pop(key)
                    tt = 3 * 4 + t4
                    nc.sync.dma_start(y_d[tt * 128:tt * 128 + 64, :], yv[0:64, :])
                    nc.gpsimd.dma_start(y_d[tt * 128 + 64:(tt + 1) * 128, :],
                                        yv[64:128, :])

            def run_filler(key, part):
                kind = key[0]
                if kind == 'qk':
                    _, m, c = key
                    if part == 0:
                        proj_qk_part(m, c, 0, NDT)
                    elif part == 1:
                        proj_qk_part(m, c, 0, 4)
                    else:
                        proj_qk_part(m, c, 4, NDT)
                elif kind == 'v':
                    _, hp, kt = key
                    proj_v2(hp, kt)
                elif kind == 'v4':
                    _, pair, kt = key
                    proj_v4(pair, kt)
                elif kind == 'outp':
                    _, qc, t4 = key
                    if part == 0:
                        out_half(qc, t4, 0)
                        out_half(qc, t4, 1)
                    elif part == 1:
                        out_half(qc, t4, 0)
                    else:
                        out_half(qc, t4, 1)
                elif kind == 'f01':
                    _, t4, c2 = key
                    out_f01(t4, c2)
                elif kind == 'f2':
                    _, t4, c2 = key
                    out_f2(t4, c2)

            # ---- prologue: just enough for the exp stream to start ----
            proj_qk_part(4, 0, 0, NDT)   # K feats for hp0, token chunk 0
            proj_qk_part(0, 0, 0, NDT)   # Q feats for qc0

            state = {}

            def emit_block_tail(bi):
                qc, hp = BLOCKS[bi]
                ops = state.pop(bi)["ops"]
                OTc = otpool.tile([128, 512], BF16, name="OTc", tag="otc")
                for sub in range(2):
                    rec = recpool.tile([64, 512], F32, name="rec", tag="rec")
                    nc.vector.reciprocal_approx_fast(rec[:], ops[sub][0:64, :])
                    nc.vector.tensor_mul(
                        OTc[sub * 64:sub * 64 + 64, :],
                        ops[sub][64:128, :], rec[:])
                otc_by_qc.setdefault(qc, {})[hp] = OTc

            def pv_pair(bi, kt, pt):
                qc, hp = BLOCKS[bi]
                ops = state[bi]["ops"]
                for sub in range(2):
                    h = 2 * hp + sub
                    nc.tensor.matmul(
                        ops[sub][:],
                        lhsT=VHAT[:, kt * VROW + h * 128: kt * VROW + (h + 1) * 128],
                        rhs=pt[:, sub * 512:(sub + 1) * 512],
                        start=(kt == 0), stop=(kt == NKT - 1))

            # ---- attention: flattened pipeline over BLOCKS x kt ----
            # Per period: lagged PV pairs, prev-block tail (at kt==2),
            # fillers, then the S^T pair LAST (so its WAR on the exp two
            # periods back is satisfied long before it reaches the PE head),
            # and the exp itself.
            pts = {}

            def emit_st(i):
                bi, kt = i // NKT, i % NKT
                qc, hp = BLOCKS[bi]
                ktf = 4 + hp
                st = psST.tile([128, 1024], F32, name="st", tag="st")
                for sub in range(2):
                    lo = sub * 64
                    nc.tensor.matmul(
                        st[:, sub * 512:(sub + 1) * 512],
                        lhsT=QTKT[lo:lo + 64, ktf * T + kt * 128: ktf * T + (kt + 1) * 128],
                        rhs=QTKT[lo:lo + 64, hp * T + qc * 512: hp * T + qc * 512 + 512],
                        start=True, stop=True)
                return st

            def emit_exp(i, st):
                bi, kt = i // NKT, i % NKT
                pt = ptpool.tile([128, 1024], BF16, name="pt", tag="pt")
                nc.scalar.activation(
                    pt[:], st[:], AF.Exp,
                    bias=BK[:, kt:kt + 1], scale=0.125)
                pts[(bi, kt)] = pt

            for i in range(NB * NKT):
                bi, kt = i // NKT, i % NKT
                if kt == 0:
                    op0 = psops.tile([128, 512], F32, name="op0", tag="ops")
                    op1 = psops.tile([128, 512], F32, name="op1", tag="ops")
                    state[bi] = {"ops": (op0, op1)}

                # the first two periods emit their S^T up front so the exp
                # stream starts before the (cold, DMA-bound) startup JIT
                if i < 2:
                    emit_exp(i, emit_st(i))

                entries = sched.get(i, ())
                for key, part in entries:
                    if key[0] == 'pv':
                        bj, ktj = divmod(key[1], NKT)
                        pv_pair(bj, ktj, pts.pop((bj, ktj)))
                if kt == 4 and bi > 0:
                    emit_block_tail(bi - 1)
                for key, part in entries:
                    if key[0] != 'pv':
                        run_filler(key, part)

                # S^T pairs run back-to-back for (even, odd) period pairs at
                # the end of the odd period: one K=64<->K=128 PE pipeline
                # boundary per two periods instead of two, and the WAR on
                # the exp two periods back is satisfied long before the
                # pair reaches the PE queue head.
                if i >= 2 and i % 2 == 1:
                    st_a = emit_st(i - 1)
                    st_b = emit_st(i)
                    emit_exp(i - 1, st_a)
                    emit_exp(i, st_b)

            # flush PV pairs scheduled beyond the last iteration
            for p in range(NB * NKT, NB * NKT + 4):
                for key, part in sched.get(p, ()):
                    if key[0] == 'pv':
                        bj, ktj = divmod(key[1], NKT)
                        pv_pair(bj, ktj, pts.pop((bj, ktj)))
                    else:
                        run_filler(key, part)
            emit_block_tail(NB - 1)
            # qc3 epilogue: only hp3's contribution + writeback remain
            k = 0
            for t4 in range(4):
                for c2 in range(2):
                    out_f3(t4, c2, k)
                    k += 1
    nc.compile()
    return nc


def get_program():
    if "nc" not in _cache:
        _cache["nc"] = _build_program()
    return _cache["nc"]


def shard_inputs(x, engagement, mask, qkv_w, qkv_b, out_w, out_b):
    """Build the per-core input maps (host-side layout prep only)."""
    x = np.asarray(x, dtype=np.float32)
    engagement = np.asarray(engagement, dtype=np.float32)
    maskf = np.asarray(mask).astype(np.float32)
    qkv_w = np.asarray(qkv_w, dtype=np.float32)
    qkv_b = np.asarray(qkv_b, dtype=np.float32)
    out_w = np.asarray(out_w, dtype=np.float32)
    out_b = np.asarray(out_b, dtype=np.float32)

    # per-key exp bias: ln(clip(eng)) - 1e9*mask, [B, T] fp32 on the host
    bk_all = np.log(np.clip(engagement, 1e-6, None)) - 1e9 * maskf

    qkvT = qkv_w.T  # [D, 3D]
    outT = out_w.T  # [D, D]
    in_maps = []
    for cix in range(8):
        b, hg = cix // 2, cix % 2
        qcols = qkvT[:, hg * 512:(hg + 1) * 512]
        kcols = qkvT[:, 1024 + hg * 512: 1024 + (hg + 1) * 512]
        sel = np.concatenate([qcols, kcols], axis=1)  # [1024 din, 1024 feats]
        # [d, p, m, f] -> [m, p, d, f] -> [(m p), (d f)]
        wqk = sel.reshape(NDT, 128, 8, 128).transpose(2, 1, 0, 3).reshape(1024, 1024)
        bq = qkv_b[hg * 512:(hg + 1) * 512].reshape(4, 128).T
        bk = qkv_b[1024 + hg * 512: 1024 + (hg + 1) * 512].reshape(4, 128).T
        bo = np.broadcast_to(out_b, (128, 1024)) if hg == 0 else np.zeros((128, 1024), np.float32)
        small1 = np.concatenate(
            [bq, bk, bk_all[b].reshape(NKT, 128).T], axis=1)
        in_maps.append({
            "xt": np.ascontiguousarray(x[b].T).astype(NP_BF16),
            "wqk": np.ascontiguousarray(wqk).astype(NP_BF16),
            "wv": np.ascontiguousarray(
                qkvT[:, 2048 + hg * 512: 2048 + (hg + 1) * 512]).astype(NP_BF16),
            "small1": np.ascontiguousarray(small1),
            "bv": np.ascontiguousarray(
                np.broadcast_to(qkv_b[2048 + hg * 512: 2048 + (hg + 1) * 512], (128, 512))),
            "wo": np.ascontiguousarray(outT[hg * 512:(hg + 1) * 512, :]).astype(NP_BF16),
            "bo": np.ascontiguousarray(bo),
        })
    return in_maps


def kernel(x, engagement, mask, qkv_w, qkv_b, out_w, out_b):
    global last_results
    nc = get_program()
    in_maps = shard_inputs(x, engagement, mask, qkv_w, qkv_b, out_w, out_b)
    res = run_bass_kernel_spmd(nc, in_maps, list(range(8)))
    last_results = res
    out = np.empty((B, T, D), dtype=np.float32)
    for b in range(B):
        out[b] = (res.results[2 * b]["y"].astype(np.float32)
                  + res.results[2 * b + 1]["y"].astype(np.float32))
    return out


# revision 43
# speedup vs baseline: 1.0206x; 1.0206x over previous
"""EngagementBiasedMHA on 8 Trainium2 NeuronCores.

Sharding: 4 batches x 2 head-groups (8 heads each).  Each core computes, for
its (batch, head-group):
  - K^T projection in [feat, token] layout and V projection in [token, feat]
    layout; V is stored per key-tile as [ones(64) | V_h] so the PV
    matmul also produces the softmax denominator on partitions 0:64
  - per 512-query chunk: attention in transposed layout: S^T = K @ Q^T with
    keys on partitions, so the per-key engagement bias/mask folds into the
    Exp activation as a per-partition bias, and exp(S^T) is already the
    correct (lhs-contraction) layout for the PV matmul
  - O^T = Vhat^T @ P^T accumulated over key tiles (rows 0:64 = replicated
    softmax denominator, rows 64:128 = head output)
  - row-parallel partial output projection y_partial = O_hg @ out_w.T[hg]
Matmul operands are bf16 (4x PE throughput vs fp32); accumulation stays fp32.

Schedule: the kernel is ACT(exp)-bound in steady state (256 exps of
[128,1024] at ~1.1us each).  All projection work except a minimal prologue
(K feats m=4 chunk 0 + Q feats m=0 chunk 0) is folded into the attention
loop as deadline-placed filler matmuls so the exp stream starts ~6us in and
the PE backfills projections in the per-period slack.  K/Q bias-adds run on
DVE (not ACT) so ACT does exps only.  Next-block S^T pairs are emitted
before evac/boundary work to avoid hp-boundary exp bubbles.

Host side: transpose/slice inputs per core, then sum the two partial outputs
per batch (row-parallel unshard).
"""

import sys

if "/opt/trn_rl_repo" not in sys.path:
    sys.path.insert(0, "/opt/trn_rl_repo")

import numpy as np
from concourse import bacc, tile
import concourse.mybir as mybir
from concourse.bass_utils import run_bass_kernel_spmd

F32 = mybir.dt.float32
BF16 = mybir.dt.bfloat16
NP_BF16 = mybir.dt.np(BF16)
AF = mybir.ActivationFunctionType

B, T, D, H = 4, 2048, 1024, 16
HD = 64
HG = 8           # heads per core
NKT = T // 128   # 16 key/token tiles
NQC = T // 512   # 4 query chunks
NDT = D // 128   # 8 d_in tiles
VROW = HG * 128  # 1024 Vhat columns per key tile: per head [ones(64) | V(64)]

_cache = {}

# Results of the most recent run (for the test harness to read exec times).
last_results = None


def _build_program():
    nc = bacc.Bacc("TRN2", target_bir_lowering=False, debug=False, num_devices=8)
    xt_d = nc.declare_dram_parameter("xt", [D, T], BF16, isOutput=False)
    # wqk: row block m*128+p holds, at col d*128+f, weight qkv_w.T[d*128+p, feat(m)+f]
    wqk_d = nc.declare_dram_parameter("wqk", [1024, 1024], BF16, isOutput=False)
    wv_d = nc.declare_dram_parameter("wv", [D, 512], BF16, isOutput=False)
    # small1 = [bqk(8) | BK(16)] merged to one DMA; BK = ln(eng)-1e9*mask (host)
    small1_d = nc.declare_dram_parameter("small1", [128, 24], F32, isOutput=False)
    bv_d = nc.declare_dram_parameter("bv", [128, 512], F32, isOutput=False)
    wo_d = nc.declare_dram_parameter("wo", [512, 1024], BF16, isOutput=False)
    bo_d = nc.declare_dram_parameter("bo", [128, 1024], F32, isOutput=False)
    y_d = nc.declare_dram_parameter("y", [T, D], BF16, isOutput=True)

    with tile.TileContext(nc) as tc:
        with (
            tc.tile_pool(name="persist", bufs=1) as persist,
            tc.tile_pool(name="wvpool", bufs=1) as wvpool,
            tc.tile_pool(name="wopool", bufs=1) as wopool,
            tc.tile_pool(name="small", bufs=1) as small,
            tc.tile_pool(name="ptpool", bufs=12) as ptpool,
            tc.tile_pool(name="otpool", bufs=9) as otpool,
            tc.tile_pool(name="evacpool", bufs=3) as evacpool,
            tc.tile_pool(name="recpool", bufs=3) as recpool,
            tc.tile_pool(name="psmix", bufs=2, space="PSUM") as psmix,
            tc.tile_pool(name="psops", bufs=2, space="PSUM") as psops,
            tc.tile_pool(name="psST", bufs=2, space="PSUM") as psST,
        ):
            # ---- resident activations / weights (bf16) ----
            XT = persist.tile([128, NDT * T], BF16, name="XT")
            WQK = persist.tile([128, 8 * 1024], BF16, name="WQK")
            WV = wvpool.tile([128, NDT * 512], BF16, name="WV")
            WO = wopool.tile([128, 4 * 1024], BF16, name="WO")
            SM1 = small.tile([128, 24], F32, name="SM1")
            DUM = small.tile([1, 8], F32, name="DUM")
            BV = small.tile([128, 512], F32, name="BV")
            BO = small.tile([128, 1024], F32, name="BO")
            QTKT = persist.tile([128, 8 * T], BF16, name="QTKT")
            VHAT = persist.tile([128, NKT * VROW], BF16, name="VHAT")

            def dma_wqk(eng, m, splits=1):
                # split across partition ranges -> parallel DMA queues
                step = 128 // splits
                for s in range(splits):
                    eng.dma_start(
                        WQK[s * step:(s + 1) * step, m * 1024:(m + 1) * 1024],
                        wqk_d[m * 128 + s * step: m * 128 + (s + 1) * step, :])

            def dma_xt_chunk(eng, c):
                for d in range(NDT):
                    eng.dma_start(
                        XT[:, d * T + c * 512: d * T + (c + 1) * 512],
                        xt_d[d * 128:(d + 1) * 128, c * 512:(c + 1) * 512])

            # Each dma_start costs ~0.7us of serialized dispatch on the
            # issuing engine's sequencer, so the input DMAs are spread over
            # three dispatch lanes (sync / gpsimd / vector), deadline-ordered
            # within each lane.
            # sync lane: all XT chunks in deadline order (they feed the
            # S-pair stream directly), then K m5 / Q m1
            nc.sync.dma_start(SM1[:], small1_d[:])
            dma_wqk(nc.sync, 4, splits=2)
            dma_wqk(nc.sync, 0, splits=2)
            dma_xt_chunk(nc.sync, 0)
            dma_xt_chunk(nc.sync, 1)
            dma_xt_chunk(nc.sync, 2)
            dma_xt_chunk(nc.sync, 3)
            dma_wqk(nc.sync, 5)
            dma_wqk(nc.sync, 1)
            BQK = SM1[:, 0:8]
            BK = SM1[:, 8:24]
            # Preload the exp table (~2.7us) off the critical path with a
            # dummy exp, and warm the PE (HAM cold throttle releases after
            # ~3.4us of activity) with ~6us of tiny matmuls so the
            # DMA-bound prologue projections run at the full 2.4 GHz.
            nc.vector.memset(DUM[:], 0.0)
            nc.scalar.activation(DUM[:], DUM[:], AF.Exp)
            for _ in range(120):
                wps = psmix.tile([8, 8], F32, name="wps", tag="mix")
                nc.tensor.matmul(wps[:], lhsT=DUM[:], rhs=DUM[:],
                                 start=True, stop=True)

            # The gpsimd DMA lane is held back behind the last xt chunk-0
            # tile so its descriptors don't contend with the prologue-
            # critical transfers on the shared DMA queues.  (Never gate the
            # scalar engine: its queue must stay clear for the exp stream.)
            GATE = small.tile([1, 8], F32, name="GATE")
            nc.gpsimd.tensor_scalar_add(GATE[0:1, 0:4], XT[0:1, 7 * T: 7 * T + 4], 0.0)

            # gpsimd lane: WV, then late weights
            for d in range(NDT):
                nc.gpsimd.dma_start(WV[:, d * 512:(d + 1) * 512],
                                    wv_d[d * 128:(d + 1) * 128, :])
            for s in range(2):
                nc.gpsimd.dma_start(BV[s * 64:(s + 1) * 64, :],
                                    bv_d[s * 64:(s + 1) * 64, :])
            dma_wqk(nc.gpsimd, 6)
            dma_wqk(nc.gpsimd, 7)
            dma_wqk(nc.gpsimd, 2)
            dma_wqk(nc.gpsimd, 3)
            for s in range(2):
                nc.gpsimd.dma_start(BO[s * 64:(s + 1) * 64, :],
                                    bo_d[s * 64:(s + 1) * 64, :])
            for f in range(4):
                nc.gpsimd.dma_start(WO[:, f * 1024:(f + 1) * 1024],
                                    wo_d[f * 128:(f + 1) * 128, :])
            # VHAT ones memsets on the vector engine
            for t in range(NKT):
                nc.vector.memset(VHAT[:, t * VROW:(t + 1) * VROW], 1.0)


            # ---- projection helpers (bias-add on DVE, not ACT) ----
            def proj_qk(m, c):
                # K feats (m=4..7) or Q feats (m=0..3) for token chunk c
                ps = psmix.tile([128, 512], F32, name="ps_qk", tag="mix")
                for d in range(NDT):
                    nc.tensor.matmul(
                        ps[:],
                        lhsT=WQK[:, m * 1024 + d * 128: m * 1024 + (d + 1) * 128],
                        rhs=XT[:, d * T + c * 512: d * T + c * 512 + 512],
                        start=(d == 0), stop=(d == NDT - 1),
                    )
                nc.vector.tensor_scalar_add(
                    QTKT[:, m * T + c * 512: m * T + c * 512 + 512],
                    ps[:], BQK[:, m:m + 1])

            def proj_v(t, pair):
                # V feats for heads [4*pair, 4*pair+4), token tile t (N=256)
                ps = psmix.tile([128, 256], F32, name="ps_v", tag="mix")
                for d in range(NDT):
                    nc.tensor.matmul(
                        ps[:],
                        lhsT=XT[:, d * T + t * 128: d * T + (t + 1) * 128],
                        rhs=WV[:, d * 512 + pair * 256: d * 512 + (pair + 1) * 256],
                        start=(d == 0), stop=(d == NDT - 1),
                    )
                vslice = VHAT[:, t * VROW + pair * 512: t * VROW + (pair + 1) * 512
                              ].rearrange("p (h c) -> p h c", c=128)[:, :, 64:128]
                nc.vector.tensor_add(
                    vslice,
                    ps[:].rearrange("p (h c) -> p h c", c=64),
                    BV[:, pair * 256:(pair + 1) * 256].rearrange(
                        "p (h c) -> p h c", c=64))

            def out_proj(qc2, otc2, grp):
                t4, c2 = grp // 2, grp % 2
                tt = qc2 * 4 + t4
                ps = psmix.tile([128, 512], F32, name="ps_y", tag="mix")
                for f in range(4):
                    nc.tensor.matmul(
                        ps[:],
                        lhsT=otc2[f][:, t4 * 128:(t4 + 1) * 128],
                        rhs=WO[:, f * 1024 + c2 * 512: f * 1024 + c2 * 512 + 512],
                        start=(f == 0), stop=(f == 3))
                yv = evacpool.tile([128, 512], BF16, name="yv", tag="yv")
                nc.vector.tensor_add(yv[:], ps[:], BO[:, c2 * 512:(c2 + 1) * 512])
                # split across two queues so the last chunk's writeback
                # doesn't leave a serial 5us DMA tail
                for s in range(2):
                    nc.sync.dma_start(
                        y_d[tt * 128 + s * 64: tt * 128 + (s + 1) * 64,
                            c2 * 512:(c2 + 1) * 512],
                        yv[s * 64:(s + 1) * 64, :])

            # ---- prologue: just enough for the exp stream to start ----
            proj_qk(4, 0)   # K feats for hp0, token chunk 0
            proj_qk(0, 0)   # Q feats for qt0, query chunk 0

            # ---- block order: interleave qc0/qc1 (then qc2/qc3) so the
            # projection fillers' deadlines spread over 128 periods instead
            # of crowding into the first 64 ----
            BLOCKS = [(0, 0), (0, 1), (1, 0), (1, 1), (0, 2), (0, 3), (1, 2), (1, 3),
                      (2, 0), (2, 1), (3, 0), (3, 1), (2, 2), (2, 3), (3, 2), (3, 3)]

            # ---- filler schedule: block index, kt -> list of thunks ----
            fillers = {}

            def add_filler(bi, kt, thunk):
                fillers.setdefault((bi, kt), []).append(thunk)

            # K feature tiles, chunk-granular just-in-time:
            #   m=4: prologue c0; c1/c2/c3 inside blk0 (first hp0 block)
            #   m=5: c0 at blk0 tail; c1/2/3 inside blk1 (first hp1 block)
            #   m=6: c0 at blk3 tail; c1/2/3 inside blk4 (first hp2 block)
            #   m=7: c0 at blk4 tail; c1/2/3 inside blk5 (first hp3 block)
            for c in range(1, 4):
                add_filler(0, 4 * c - 3, (lambda c=c: proj_qk(4, c)))
                add_filler(1, 4 * c - 3, (lambda c=c: proj_qk(5, c)))
                add_filler(4, 4 * c - 3, (lambda c=c: proj_qk(6, c)))
                add_filler(5, 4 * c - 3, (lambda c=c: proj_qk(7, c)))
            add_filler(0, 13, (lambda: proj_qk(5, 0)))
            add_filler(3, 13, (lambda: proj_qk(6, 0)))
            add_filler(4, 13, (lambda: proj_qk(7, 0)))
            # Q-tiles JIT: block bi needs Q(m=hp, c=qc); emit one block ahead
            for bi in range(1, 16):
                qc, hp = BLOCKS[bi]
                add_filler(bi - 1, 14, (lambda hp=hp, qc=qc: proj_qk(hp, qc)))
            # V projection: pair0 (heads 0..3) during blk0, pair1 during blk4
            for t in range(NKT):
                add_filler(0, t, (lambda t=t: proj_v(t, 0)))
                add_filler(4, t, (lambda t=t: proj_v(t, 1)))

            # deferred out-proj groups: qc ready after its last block's evac
            # (qc0 after blk5, qc1 after blk7, qc2 after blk13, qc3 at end).
            # Groups are spread across the NEXT block's kts as fillers so the
            # boundary never dumps a multi-us matmul burst in front of the
            # S-pairs (which would drain the 2-deep st buffer and stall ACT).
            outproj_spread = {
                6: [(0, 0), (0, 1)], 7: [(0, 2), (0, 3)], 8: [(0, 4), (0, 5)],
                9: [(0, 6), (0, 7)],
                10: [(1, 0), (1, 1)], 11: [(1, 2), (1, 3)], 12: [(1, 4), (1, 5)],
                13: [(1, 6), (1, 7), (2, 0), (2, 1)],
                14: [(2, 2), (2, 3), (2, 4), (2, 5), (2, 6), (2, 7)],
            }
            _slots = {2: (2, 8), 4: (2, 5, 8, 11), 6: (2, 4, 6, 8, 10, 12)}
            for bi, work in outproj_spread.items():
                for j, (oqc, g) in enumerate(work):
                    add_filler(bi + 1, _slots[len(work)][j],
                               (lambda oqc=oqc, g=g: out_proj(
                                   oqc, [otc_by_qc[oqc][f] for f in range(4)], g)))

            state = {}
            otc_by_qc = {}

            def emit_block_tail(bi):
                qc, hp = BLOCKS[bi]
                ops = state.pop(bi)["ops"]
                OTc = otpool.tile([128, 512], BF16, name="OTc", tag="otc")
                for sub in range(2):
                    rec = recpool.tile([64, 512], F32, name="rec", tag="rec")
                    nc.vector.reciprocal_approx_fast(rec[:], ops[sub][0:64, :])
                    nc.vector.tensor_mul(
                        OTc[sub * 64:sub * 64 + 64, :],
                        ops[sub][64:128, :], rec[:])
                otc_by_qc.setdefault(qc, {})[hp] = OTc

            def pv_pair(bi, kt, pt):
                qc, hp = BLOCKS[bi]
                ops = state[bi]["ops"]
                for sub in range(2):
                    h = 2 * hp + sub
                    nc.tensor.matmul(
                        ops[sub][:],
                        lhsT=VHAT[:, kt * VROW + h * 128: kt * VROW + (h + 1) * 128],
                        rhs=pt[:, sub * 512:(sub + 1) * 512],
                        start=(kt == 0), stop=(kt == NKT - 1))

            # ---- attention: flattened pipeline over BLOCKS x kt ----
            # Per index: S-pair, exp, prev-block tail (at kt==1), fillers,
            # then the PV pair LAGGED by one iteration so it never blocks
            # the in-order PE queue waiting on the exp or the evac.
            pv_sched = {}
            _pv_slots = (4, 5, 6, 7, 8, 8, 9, 10, 11, 12, 12, 13, 14, 15, 16, 17)
            for b in range(len(BLOCKS)):
                for k, s in zip(range(NKT), _pv_slots):
                    pv_sched.setdefault(b * NKT + s, []).append(b * NKT + k)

            pts = {}
            for i in range(len(BLOCKS) * NKT):
                bi, kt = i // NKT, i % NKT
                qc, hp = BLOCKS[bi]
                qt = hp
                ktf = 4 + hp
                if kt == 0:
                    op0 = psops.tile([128, 512], F32, name="op0", tag="ops")
                    op1 = psops.tile([128, 512], F32, name="op1", tag="ops")
                    state[bi] = {"ops": (op0, op1)}

                # PV pairs lagged per-schedule: a block's first PVs wait
                # until kt4 so they never queue behind the previous block's
                # 2.7us serial evac chain (they need the psops slots it
                # reads); two catch-up doubles mid-block restore lag-2 by
                # the block end.
                for j in pv_sched.get(i, ()):
                    bj, ktj = divmod(j, NKT)
                    pv_pair(bj, ktj, pts.pop((bj, ktj)))

                # S^T pair (the two K=64 matmuls run concurrently via
                # base_partition-derived PE row tiling)
                st = psST.tile([128, 1024], F32, name="st", tag="st")
                for sub in range(2):
                    lo = sub * 64
                    nc.tensor.matmul(
                        st[:, sub * 512:(sub + 1) * 512],
                        lhsT=QTKT[lo:lo + 64, ktf * T + kt * 128: ktf * T + (kt + 1) * 128],
                        rhs=QTKT[lo:lo + 64, qt * T + qc * 512: qt * T + qc * 512 + 512],
                        start=True, stop=True)
                pt = ptpool.tile([128, 1024], BF16, name="pt", tag="pt")
                nc.scalar.activation(
                    pt[:], st[:], AF.Exp,
                    bias=BK[:, kt:kt + 1], scale=0.125)
                pts[(bi, kt)] = pt

                if kt == 2 and bi > 0:
                    emit_block_tail(bi - 1)
                for th in fillers.get((bi, kt), ()):
                    th()

            # flush PV pairs scheduled beyond the last iteration
            for it in sorted(k for k in pv_sched if k >= len(BLOCKS) * NKT):
                for j in pv_sched[it]:
                    bj, ktj = divmod(j, NKT)
                    pv_pair(bj, ktj, pts.pop((bj, ktj)))
            emit_block_tail(15)
            # qc3's out-projs drain at the end
            otc3 = [otc_by_qc[3][f] for f in range(4)]
            for grp in range(8):
                out_proj(3, otc3, grp)
    nc.compile()
    return nc


def get_program():
    if "nc" not in _cache:
        _cache["nc"] = _build_program()
    return _cache["nc"]


def shard_inputs(x, engagement, mask, qkv_w, qkv_b, out_w, out_b):
    """Build the per-core input maps (host-side layout prep only)."""
    x = np.asarray(x, dtype=np.float32)
    engagement = np.asarray(engagement, dtype=np.float32)
    maskf = np.asarray(mask).astype(np.float32)
    qkv_w = np.asarray(qkv_w, dtype=np.float32)
    qkv_b = np.asarray(qkv_b, dtype=np.float32)
    out_w = np.asarray(out_w, dtype=np.float32)
    out_b = np.asarray(out_b, dtype=np.float32)

    # per-key exp bias: ln(clip(eng)) - 1e9*mask, [B, T] fp32 on the host
    bk_all = np.log(np.clip(engagement, 1e-6, None)) - 1e9 * maskf

    qkvT = qkv_w.T  # [D, 3D]
    outT = out_w.T  # [D, D]
    in_maps = []
    for cix in range(8):
        b, hg = cix // 2, cix % 2
        qcols = qkvT[:, hg * 512:(hg + 1) * 512]
        kcols = qkvT[:, 1024 + hg * 512: 1024 + (hg + 1) * 512]
        sel = np.concatenate([qcols, kcols], axis=1)  # [1024 din, 1024 feats]
        # [d, p, m, f] -> [m, p, d, f] -> [(m p), (d f)]
        wqk = sel.reshape(NDT, 128, 8, 128).transpose(2, 1, 0, 3).reshape(1024, 1024)
        bq = qkv_b[hg * 512:(hg + 1) * 512].reshape(4, 128).T
        bk = qkv_b[1024 + hg * 512: 1024 + (hg + 1) * 512].reshape(4, 128).T
        bo = np.broadcast_to(out_b, (128, 1024)) if hg == 0 else np.zeros((128, 1024), np.float32)
        small1 = np.concatenate(
            [bq, bk, bk_all[b].reshape(NKT, 128).T], axis=1)
        in_maps.append({
            "xt": np.ascontiguousarray(x[b].T).astype(NP_BF16),
            "wqk": np.ascontiguousarray(wqk).astype(NP_BF16),
            "wv": np.ascontiguousarray(
                qkvT[:, 2048 + hg * 512: 2048 + (hg + 1) * 512]).astype(NP_BF16),
            "small1": np.ascontiguousarray(small1),
            "bv": np.ascontiguousarray(
                np.broadcast_to(qkv_b[2048 + hg * 512: 2048 + (hg + 1) * 512], (128, 512))),
            "wo": np.ascontiguousarray(outT[hg * 512:(hg + 1) * 512, :]).astype(NP_BF16),
            "bo": np.ascontiguousarray(bo),
        })
    return in_maps


def kernel(x, engagement, mask, qkv_w, qkv_b, out_w, out_b):
    global last_results
    nc = get_program()
    in_maps = shard_inputs(x, engagement, mask, qkv_w, qkv_b, out_w, out_b)
    res = run_bass_kernel_spmd(nc, in_maps, list(range(8)))
    last_results = res
    out = np.empty((B, T, D), dtype=np.float32)
    for b in range(B):
        out[b] = (res.results[2 * b]["y"].astype(np.float32)
                  + res.results[2 * b + 1]["y"].astype(np.float32))
    return out

